# revision 5
# baseline (speedup 1.0000x reference)
"""Mixtral top-2 MoE MLP (grouped GEMM) on 8 TRN2 NeuronCores.

Strategy: host-side routing. Sort the M*K (token, expert) assignments by
expert, pad each expert's row count to a multiple of 8*128, and give every
core exactly 1/8 of every expert's rows. Each core runs the same program
(per-expert row counts are compile-time constants, identical across cores):

  for each expert e:
    phase1: hT[f, rows]   = w1[e].T @ x  and  w3[e].T @ x   (psum, 16-deep k-acc)
    gate:   hg = silu(h1) * h3                               (ACT + DVE, psum->sbuf)
    phase3: outT[hid, rows] = w2[e].T @ hg                   (psum, 32-deep f-acc)

Matmuls run in bf16 (inputs pre-cast on host); accumulation is fp32 in PSUM.
No collectives: each core owns disjoint rows. Host scatters outputs back.
"""

import numpy as np
import ml_dtypes

M, TOPK, HID, FFN, E = 32768, 2, 2048, 4096, 8
NCORES = 8
KT = HID // 128          # 16 k-tiles
FT = FFN // 128          # 32 f-tiles
NHT = HID // 128         # 16 hid-tiles (phase3 out partition tiles)
CHUNK = 512              # moving-dim rows per matmul
ROW_PAD = NCORES * 128   # per-expert global row padding granularity

BF16 = ml_dtypes.bfloat16


def _build_routing(top_ks: np.ndarray):
    """Returns per-core (col_idx[rows_pc] token ids, dest_idx[rows_pc] flat
    assignment ids or -1, n_e[E] per-core per-expert row counts)."""
    top_flat = top_ks.reshape(-1)
    order = np.argsort(top_flat, kind="stable")          # assignment ids, sorted by expert
    counts = np.bincount(top_flat, minlength=E)
    tok_of_assign = order // TOPK                        # token id per sorted row

    core_cols = [[] for _ in range(NCORES)]
    core_dest = [[] for _ in range(NCORES)]
    n_e = []
    pos = 0
    for e in range(E):
        c = int(counts[e])
        assign_e = order[pos:pos + c]
        tok_e = tok_of_assign[pos:pos + c]
        pos += c
        padded = ((c + ROW_PAD - 1) // ROW_PAD) * ROW_PAD if c > 0 else 0
        per_core = padded // NCORES
        n_e.append(per_core)
        if padded == 0:
            continue
        pad = padded - c
        tok_pad = np.concatenate([tok_e, np.zeros(pad, np.int64)])
        dest_pad = np.concatenate([assign_e, np.full(pad, -1, np.int64)])
        for cc in range(NCORES):
            core_cols[cc].append(tok_pad[cc * per_core:(cc + 1) * per_core])
            core_dest[cc].append(dest_pad[cc * per_core:(cc + 1) * per_core])
    col_idx = [np.concatenate(core_cols[c]) for c in range(NCORES)]
    dest_idx = [np.concatenate(core_dest[c]) for c in range(NCORES)]
    return col_idx, dest_idx, n_e


def _chunks(n):
    """Split n rows into chunks of <=CHUNK, multiples of 128."""
    out = []
    while n > 0:
        c = min(CHUNK, n)
        out.append(c)
        n -= c
    return out


def _build_program(n_e, rows_pc):
    import concourse.bacc as bacc
    import concourse.tile as tile
    import concourse.mybir as mybir

    nc = bacc.Bacc("TRN2", target_bir_lowering=False, debug=False,
                   enable_asserts=False, num_devices=NCORES)
    dtb = mybir.dt.bfloat16
    xT_d = nc.dram_tensor("xT", [HID, rows_pc], dtb, kind="ExternalInput").ap()
    w1_d = nc.dram_tensor("w1", [E, HID, FFN], dtb, kind="ExternalInput").ap()
    w2_d = nc.dram_tensor("w2", [E, FFN, HID], dtb, kind="ExternalInput").ap()
    w3_d = nc.dram_tensor("w3", [E, HID, FFN], dtb, kind="ExternalInput").ap()
    oT_d = nc.dram_tensor("oT", [HID, rows_pc], mybir.dt.float32,
                          kind="ExternalOutput").ap()

    # work items (expert, n_rows<=1536) so phase3 psum needs <=3 chunk banks
    work = []
    for e in range(E):
        ne = n_e[e]
        while ne > 0:
            take = min(ne, 3 * CHUNK)
            work.append((e, take))
            ne -= take
    max_ne = max(r for _, r in work)
    with tile.TileContext(nc) as tc:
        with tc.tile_pool(name="xp", bufs=1) as xp, \
             tc.tile_pool(name="wp", bufs=4) as wp, \
             tc.tile_pool(name="w2p", bufs=4) as w2p, \
             tc.tile_pool(name="hgp", bufs=1) as hgp, \
             tc.tile_pool(name="gtp", bufs=4) as gtp, \
             tc.tile_pool(name="osp", bufs=4) as osp, \
             tc.tile_pool(name="ps", bufs=1, space="PSUM") as ps:
            col0 = 0
            for wi, (e, ne) in enumerate(work):
                chs = _chunks(ne)
                # ---- load xT for this work item: [128, KT, ne] (bf16)
                xTe = xp.tile([128, KT, max_ne], dtb, name=f"xTe{wi}", tag="xTe")
                for t in range(KT):
                    nc.sync.dma_start(
                        xTe[:, t, :ne],
                        xT_d[t * 128:(t + 1) * 128, col0:col0 + ne])
                # ---- phase 1 + gate: hg[f, rows] = silu(x@w1)*(x@w3), T layout
                hg = hgp.tile([128, FT, max_ne], dtb, name=f"hg{wi}", tag="hg")
                for ft in range(FT):
                    wst1 = wp.tile([128, KT, 128], dtb, name=f"w1s{e}_{ft}", tag="w1s")
                    nc.sync.dma_start(
                        wst1[:],
                        w1_d[e, :, ft * 128:(ft + 1) * 128]
                        .rearrange("(t p) f -> p t f", p=128))
                    wst3 = wp.tile([128, KT, 128], dtb, name=f"w3s{e}_{ft}", tag="w3s")
                    nc.sync.dma_start(
                        wst3[:],
                        w3_d[e, :, ft * 128:(ft + 1) * 128]
                        .rearrange("(t p) f -> p t f", p=128))
                    off = 0
                    for ci, ch in enumerate(chs):
                        ph1 = ps.tile([128, CHUNK], mybir.dt.float32,
                                      name=f"ph1_{e}_{ft}_{ci}", tag=f"ph1_{ci % 2}")
                        ph3 = ps.tile([128, CHUNK], mybir.dt.float32,
                                      name=f"ph3_{e}_{ft}_{ci}", tag=f"ph3_{ci % 2}")
                        for t in range(KT):
                            nc.tensor.matmul(ph1[:, :ch], lhsT=wst1[:, t, :],
                                             rhs=xTe[:, t, off:off + ch],
                                             start=(t == 0), stop=(t == KT - 1))
                        for t in range(KT):
                            nc.tensor.matmul(ph3[:, :ch], lhsT=wst3[:, t, :],
                                             rhs=xTe[:, t, off:off + ch],
                                             start=(t == 0), stop=(t == KT - 1))
                        sil = gtp.tile([128, CHUNK], mybir.dt.float32,
                                       name=f"sil{e}_{ft}_{ci}", tag=f"sil{ci % 2}")
                        nc.scalar.activation(sil[:, :ch], ph1[:, :ch],
                                             mybir.ActivationFunctionType.Silu)
                        nc.vector.tensor_mul(hg[:, ft, off:off + ch],
                                             sil[:, :ch], ph3[:, :ch])
                        off += ch
                # ---- phase 3: outT[hid, rows] = w2[e].T @ hg
                for m in range(NHT):
                    w2st = w2p.tile([128, FT, 128], dtb, name=f"w2s{e}_{m}", tag="w2s")
                    nc.sync.dma_start(
                        w2st[:],
                        w2_d[e, :, m * 128:(m + 1) * 128]
                        .rearrange("(t p) h -> p t h", p=128))
                    pos = [ps.tile([128, CHUNK], mybir.dt.float32,
                                   name=f"po_{wi}_{m}_{ci}", tag=f"po_{ci}")
                           for ci in range(len(chs))]
                    for ft in range(FT):
                        off = 0
                        for ci, ch in enumerate(chs):
                            nc.tensor.matmul(pos[ci][:, :ch], lhsT=w2st[:, ft, :],
                                             rhs=hg[:, ft, off:off + ch],
                                             start=(ft == 0), stop=(ft == FT - 1))
                            off += ch
                    off = 0
                    for ci, ch in enumerate(chs):
                        ost = osp.tile([128, CHUNK], mybir.dt.float32,
                                       name=f"ost{e}_{m}_{ci}", tag=f"ost{ci % 2}")
                        nc.vector.tensor_copy(ost[:, :ch], pos[ci][:, :ch])
                        nc.sync.dma_start(
                            oT_d[m * 128:(m + 1) * 128, col0 + off:col0 + off + ch],
                            ost[:, :ch])
                        off += ch
                col0 += ne
    nc.compile()
    return nc


def kernel(hidden_states, top_ks, w1, w2, w3):
    from concourse import bass_utils

    hidden_states = np.asarray(hidden_states)
    top_ks = np.asarray(top_ks)
    col_idx, dest_idx, n_e = _build_routing(top_ks)
    rows_pc = int(sum(n_e))

    nc = _build_program(n_e, rows_pc)

    hb = hidden_states.astype(BF16)
    w1b = np.asarray(w1).astype(BF16)
    w2b = np.asarray(w2).astype(BF16)
    w3b = np.asarray(w3).astype(BF16)

    in_maps = []
    for c in range(NCORES):
        xT_c = np.ascontiguousarray(hb[col_idx[c]].T)     # [HID, rows_pc] bf16
        in_maps.append({"xT": xT_c, "w1": w1b, "w2": w2b, "w3": w3b})

    res = bass_utils.run_bass_kernel_spmd(nc, in_maps, core_ids=list(range(NCORES)))

    out_flat = np.zeros((M * TOPK, HID), np.float32)
    for c in range(NCORES):
        oT = res.results[c]["oT"]                          # [HID, rows_pc] fp32
        d = dest_idx[c]
        valid = d >= 0
        out_flat[d[valid]] = oT.T[valid]
    return out_flat.reshape(M, TOPK, HID)


# revision 14
# speedup vs baseline: 11204.7689x; 11204.7689x over previous
"""Mixtral top-2 MoE MLP (grouped GEMM) on 8 TRN2 NeuronCores.

Strategy: host-side routing. Sort the M*K (token, expert) assignments by
expert, pad each expert's row count to a multiple of 8*128, and give every
core exactly 1/8 of every expert's rows. Each core runs the same program
(per-expert row counts are compile-time constants, identical across cores):

  for each expert e:
    phase1: hT[f, rows]   = w1[e].T @ x  and  w3[e].T @ x   (psum, 16-deep k-acc)
    gate:   hg = silu(h1) * h3                               (ACT + DVE, psum->sbuf)
    phase3: outT[hid, rows] = w2[e].T @ hg                   (psum, 32-deep f-acc)

Matmuls run in bf16 (inputs pre-cast on host); accumulation is fp32 in PSUM.
No collectives: each core owns disjoint rows. Host scatters outputs back.
"""

import numpy as np
import ml_dtypes

M, TOPK, HID, FFN, E = 32768, 2, 2048, 4096, 8
NCORES = 8
KT = HID // 128          # 16 k-tiles
FT = FFN // 128          # 32 f-tiles
NHT = HID // 128         # 16 hid-tiles (phase3 out partition tiles)
CHUNK = 512              # moving-dim rows per matmul
ROW_PAD = NCORES * 128   # per-expert global row padding granularity

BF16 = ml_dtypes.bfloat16


def _build_routing(top_ks: np.ndarray):
    """Returns per-core (col_idx[rows_pc] token ids, dest_idx[rows_pc] flat
    assignment ids or -1, n_e[E] per-core per-expert row counts, dup_m).

    Tokens whose two experts coincide are computed once (slot 0); the host
    copies slot 0 -> slot 1 afterwards (dup_m lists those tokens)."""
    top_flat = top_ks.reshape(-1)
    dup_m = np.nonzero(top_ks[:, 0] == top_ks[:, 1])[0]
    keep = np.ones(top_flat.shape[0], bool)
    keep[dup_m * TOPK + 1] = False
    idx_keep = np.nonzero(keep)[0]
    sub_order = np.argsort(top_flat[idx_keep], kind="stable")
    order = idx_keep[sub_order]                          # assignment ids, sorted by expert
    counts = np.bincount(top_flat[idx_keep], minlength=E)
    tok_of_assign = order // TOPK                        # token id per sorted row

    core_cols = [[] for _ in range(NCORES)]
    core_dest = [[] for _ in range(NCORES)]
    n_e = []
    pos = 0
    for e in range(E):
        c = int(counts[e])
        assign_e = order[pos:pos + c]
        tok_e = tok_of_assign[pos:pos + c]
        pos += c
        padded = ((c + ROW_PAD - 1) // ROW_PAD) * ROW_PAD if c > 0 else 0
        per_core = padded // NCORES
        n_e.append(per_core)
        if padded == 0:
            continue
        pad = padded - c
        tok_pad = np.concatenate([tok_e, np.zeros(pad, np.int64)])
        dest_pad = np.concatenate([assign_e, np.full(pad, -1, np.int64)])
        for cc in range(NCORES):
            core_cols[cc].append(tok_pad[cc * per_core:(cc + 1) * per_core])
            core_dest[cc].append(dest_pad[cc * per_core:(cc + 1) * per_core])
    col_idx = [np.concatenate(core_cols[c]) for c in range(NCORES)]
    dest_idx = [np.concatenate(core_dest[c]) for c in range(NCORES)]
    return col_idx, dest_idx, n_e, dup_m


def _chunks(n):
    """Split n rows into chunks of <=CHUNK, multiples of 128."""
    out = []
    while n > 0:
        c = min(CHUNK, n)
        out.append(c)
        n -= c
    return out


def _build_program(n_e, rows_pc, repeat=1):
    import concourse.bacc as bacc
    import concourse.tile as tile
    import concourse.mybir as mybir

    nc = bacc.Bacc("TRN2", target_bir_lowering=False, debug=False,
                   enable_asserts=False, num_devices=NCORES)
    dtb = mybir.dt.bfloat16
    xT_d = nc.dram_tensor("xT", [HID, rows_pc], dtb, kind="ExternalInput").ap()
    w1_d = nc.dram_tensor("w1", [E, HID, FFN], dtb, kind="ExternalInput").ap()
    w2_d = nc.dram_tensor("w2", [E, FFN, HID], dtb, kind="ExternalInput").ap()
    w3_d = nc.dram_tensor("w3", [E, HID, FFN], dtb, kind="ExternalInput").ap()
    oT_d = nc.dram_tensor("oT", [HID, rows_pc], mybir.dt.float32,
                          kind="ExternalOutput").ap()

    # work items (expert, n_rows<=1536) so phase3 psum needs <=3 chunk banks
    work = []
    for e in range(E):
        ne = n_e[e]
        while ne > 0:
            take = min(ne, 3 * CHUNK)
            work.append((e, take))
            ne -= take
    max_ne = max(r for _, r in work)
    offs = []
    c0 = 0
    for _, r in work:
        offs.append(c0)
        c0 += r
    # timing only: re-run the whole job `repeat` times (same col offsets)
    work = work * repeat
    offs = offs * repeat
    with tile.TileContext(nc) as tc:
        with tc.tile_pool(name="xp", bufs=1) as xp, \
             tc.tile_pool(name="wp", bufs=4) as wp, \
             tc.tile_pool(name="w2p", bufs=4) as w2p, \
             tc.tile_pool(name="hgp", bufs=1) as hgp, \
             tc.tile_pool(name="gtp", bufs=4) as gtp, \
             tc.tile_pool(name="osp", bufs=4) as osp, \
             tc.tile_pool(name="ps", bufs=1, space="PSUM") as ps:
            for wi, (e, ne) in enumerate(work):
                col0 = offs[wi]
                chs = _chunks(ne)
                # ---- load xT for this work item: [128, KT, ne] (bf16)
                xTe = xp.tile([128, KT, max_ne], dtb, name=f"xTe{wi}", tag="xTe")
                for t in range(KT):
                    nc.sync.dma_start(
                        xTe[:, t, :ne],
                        xT_d[t * 128:(t + 1) * 128, col0:col0 + ne])
                # ---- phase 1 + gate: hg[f, rows] = silu(x@w1)*(x@w3), T layout
                hg = hgp.tile([128, FT, max_ne], dtb, name=f"hg{wi}", tag="hg")
                for ft in range(FT):
                    wst1 = wp.tile([128, KT, 128], dtb, name=f"w1s{e}_{ft}", tag="w1s")
                    nc.sync.dma_start(
                        wst1[:],
                        w1_d[e, :, ft * 128:(ft + 1) * 128]
                        .rearrange("(t p) f -> p t f", p=128))
                    wst3 = wp.tile([128, KT, 128], dtb, name=f"w3s{e}_{ft}", tag="w3s")
                    nc.sync.dma_start(
                        wst3[:],
                        w3_d[e, :, ft * 128:(ft + 1) * 128]
                        .rearrange("(t p) f -> p t f", p=128))
                    off = 0
                    for ci, ch in enumerate(chs):
                        ph1 = ps.tile([128, CHUNK], mybir.dt.float32,
                                      name=f"ph1_{e}_{ft}_{ci}", tag=f"ph1_{ci % 2}")
                        ph3 = ps.tile([128, CHUNK], mybir.dt.float32,
                                      name=f"ph3_{e}_{ft}_{ci}", tag=f"ph3_{ci % 2}")
                        for t in range(KT):
                            nc.tensor.matmul(ph1[:, :ch], lhsT=wst1[:, t, :],
                                             rhs=xTe[:, t, off:off + ch],
                                             start=(t == 0), stop=(t == KT - 1))
                        for t in range(KT):
                            nc.tensor.matmul(ph3[:, :ch], lhsT=wst3[:, t, :],
                                             rhs=xTe[:, t, off:off + ch],
                                             start=(t == 0), stop=(t == KT - 1))
                        sil = gtp.tile([128, CHUNK], mybir.dt.float32,
                                       name=f"sil{e}_{ft}_{ci}", tag=f"sil{ci % 2}")
                        nc.scalar.activation(sil[:, :ch], ph1[:, :ch],
                                             mybir.ActivationFunctionType.Silu)
                        nc.vector.tensor_mul(hg[:, ft, off:off + ch],
                                             sil[:, :ch], ph3[:, :ch])
                        off += ch
                # ---- phase 3: outT[hid, rows] = w2[e].T @ hg
                for m in range(NHT):
                    w2st = w2p.tile([128, FT, 128], dtb, name=f"w2s{e}_{m}", tag="w2s")
                    nc.sync.dma_start(
                        w2st[:],
                        w2_d[e, :, m * 128:(m + 1) * 128]
                        .rearrange("(t p) h -> p t h", p=128))
                    pos = [ps.tile([128, CHUNK], mybir.dt.float32,
                                   name=f"po_{wi}_{m}_{ci}", tag=f"po_{ci}")
                           for ci in range(len(chs))]
                    for ft in range(FT):
                        off = 0
                        for ci, ch in enumerate(chs):
                            nc.tensor.matmul(pos[ci][:, :ch], lhsT=w2st[:, ft, :],
                                             rhs=hg[:, ft, off:off + ch],
                                             start=(ft == 0), stop=(ft == FT - 1))
                            off += ch
                    off = 0
                    for ci, ch in enumerate(chs):
                        ost = osp.tile([128, CHUNK], mybir.dt.float32,
                                       name=f"ost{e}_{m}_{ci}", tag=f"ost{ci % 2}")
                        nc.vector.tensor_copy(ost[:, :ch], pos[ci][:, :ch])
                        nc.sync.dma_start(
                            oT_d[m * 128:(m + 1) * 128, col0 + off:col0 + off + ch],
                            ost[:, :ch])
                        off += ch
    nc.compile()
    return nc


def kernel(hidden_states, top_ks, w1, w2, w3):
    from concourse import bass_utils

    hidden_states = np.asarray(hidden_states)
    top_ks = np.asarray(top_ks)
    col_idx, dest_idx, n_e, dup_m = _build_routing(top_ks)
    rows_pc = int(sum(n_e))

    nc = _build_program(n_e, rows_pc)

    hb = hidden_states.astype(BF16)
    w1b = np.asarray(w1).astype(BF16)
    w2b = np.asarray(w2).astype(BF16)
    w3b = np.asarray(w3).astype(BF16)

    in_maps = []
    for c in range(NCORES):
        xT_c = np.ascontiguousarray(hb[col_idx[c]].T)     # [HID, rows_pc] bf16
        in_maps.append({"xT": xT_c, "w1": w1b, "w2": w2b, "w3": w3b})

    res = bass_utils.run_bass_kernel_spmd(nc, in_maps, core_ids=list(range(NCORES)))

    out_flat = np.zeros((M * TOPK, HID), np.float32)
    for c in range(NCORES):
        oT = res.results[c]["oT"]                          # [HID, rows_pc] fp32
        d = dest_idx[c]
        valid = d >= 0
        out_flat[d[valid]] = oT.T[valid]
    out_flat[dup_m * TOPK + 1] = out_flat[dup_m * TOPK]    # e0==e1 tokens
    return out_flat.reshape(M, TOPK, HID)
